# revision 22
# baseline (speedup 1.0000x reference)
"""AttentionBlock (GroupNorm -> qkv -> 4-head attention -> proj -> residual)
on 8 TRN2 NeuronCores.

Sharding: each core owns (batch b = core//2, query-half qh = core%2):
all 4 heads, 2048 of the 4096 query positions, full keys/values.
The host rotates x[b] along the spatial axis per core so every core's
query block is columns [0, 2048) -> one identical SPMD graph, no
collectives, host does only concat/reshape.

Per-core graph:
  GroupNorm (DVE stats + PE cross-partition group reduce, fp32)
  qkv matmuls in bf16 (q,k row-major; v produced directly transposed)
  attention with PE array packing:
    qk: 2 heads/instr-pair via 64x128 row tiles (contract=hd=64)
    av: 4 heads concurrent via 128x32 col tiles; head dims split across
        two psum banks so partition i of each bank belongs to head i//32
    den: ones[128,32] lhsT broadcasts each head's softmax denominator to
        its 32-partition block of a third bank
  exp split across engines: heads 0,1 exact exp on ScalarE; heads 2,3
  Schraudolph exp2 (DVE fma -> int32, GpSimd bitcast copy -> bf16)
  normalize: one approx-reciprocal + two tensor_muls per query block
  proj with host-permuted weight rows + bias + residual, DMA out
"""

import sys

import numpy as np

sys.path.insert(0, "/opt/trn_rl_repo")

import concourse.bass as bass  # noqa: E402
import concourse.tile as tile  # noqa: E402
from concourse import mybir  # noqa: E402

F32 = mybir.dt.float32
I32 = mybir.dt.int32
I16 = mybir.dt.int16
BF16 = mybir.dt.bfloat16
AF = mybir.ActivationFunctionType
OP = mybir.AluOpType
AX = mybir.AxisListType

B, C, N = 4, 256, 4096
NH, HD, G = 4, 64, 8
EPS = 1e-5
SCALE = float(HD) ** -0.5
NQ = 2048  # queries per core
NCORES = 8
CT = 2  # 128-partition tiles covering C=256
NMT = N // 128  # 32 key tiles
NQB = NQ // 512  # 4 query blocks

# Schraudolph exp2 in bf16 bits: exp(s*SCALE) ~= bitcast_bf16(round_i16(s*EA + EB))
# (int16 result is exactly the bf16 bit pattern; one DVE instruction)
EA = float(SCALE * 1.4426950408889634 * (1 << 7))
EB = float(127 * (1 << 7) - 366392.0 / 65536.0)
DVE_SCH = True
DEBUG_DUMP = False


def _body(tc, ext):
    nc = tc.nc
    from contextlib import ExitStack

    with ExitStack() as es:
        const = es.enter_context(tc.tile_pool(name="const", bufs=1))
        stage = es.enter_context(tc.tile_pool(name="stage", bufs=2))
        work = es.enter_context(tc.tile_pool(name="work", bufs=1))
        pp = es.enter_context(tc.tile_pool(name="pp", bufs=2))
        outp = es.enter_context(tc.tile_pool(name="outp", bufs=3))
        # PSUM budget (8 banks): scores sA/sB [128,512] double-buffered (4)
        # + av0/av1 dim-half accumulators (2, column-alternated across passes;
        # the two dim-half groups are simultaneously open so they must live in
        # different banks - one open accumulation group per bank row)
        # + den [128,512] column-alternated (1) + qkv producers (1)
        ps_sc = es.enter_context(tc.tile_pool(name="ps_sc", bufs=2, space="PSUM"))
        ps_av = es.enter_context(tc.tile_pool(name="ps_av", bufs=1, space="PSUM"))
        ps_acc = es.enter_context(tc.tile_pool(name="ps_acc", bufs=1, space="PSUM"))
        ps_bank = es.enter_context(tc.tile_pool(name="ps_bank", bufs=1, space="PSUM"))

        # ---------------- input DMA + weight casts ----------------
        xt = [work.tile([128, N], F32, tag=f"x{t}", name=f"x{t}") for t in range(CT)]
        for t in range(CT):
            nc.sync.dma_start(out=xt[t][:], in_=ext["x"][128 * t : 128 * (t + 1), :])

        # Small constants: DMA into raw staging tiles, then DVE-copy into
        # per-use tiles, so every downstream consumer depends on the DVE
        # semaphore only (walrus caps sync waits per instruction).
        qb_b, kb_b, gnw, gnb, projb = [], [], [], [], []
        braw = stage.tile([128, 16], F32, tag="braw", name="braw")
        vraw = stage.tile([1, 256], F32, tag="vraw", name="vraw")
        iraw = stage.tile([128, 4], F32, tag="iraw", name="iraw")
        traw = stage.tile([4, 128], F32, tag="traw", name="traw")
        col = 0
        dmas = []
        for t in range(CT):
            for lst, src_ap in (
                (qb_b, ext["qkv_b"][t]),
                (kb_b, ext["qkv_b"][2 + t]),
                (gnw, ext["gn_w"][t]),
                (gnb, ext["gn_b"][t]),
                (projb, ext["proj_b"][t]),
            ):
                nc.sync.dma_start(out=braw[:, col : col + 1], in_=src_ap)
                dmas.append((lst, col))
                col += 1
        nc.sync.dma_start(out=vraw[:], in_=ext["vb"][:])
        nc.sync.dma_start(out=iraw[:], in_=ext["ind128"][:])
        nc.sync.dma_start(out=traw[:], in_=ext["indT"][:])
        for lst, cl in dmas:
            tl = const.tile([128, 1], F32, tag=f"bc{cl}", name=f"bc{cl}")
            nc.vector.tensor_copy(tl[:], braw[:, cl : cl + 1])
            lst.append(tl)
        vb = const.tile([1, C], F32, tag="vb", name="vb")
        nc.vector.tensor_copy(vb[:], vraw[:])
        ind128 = const.tile([128, 4], F32, tag="ind128", name="ind128")
        nc.vector.tensor_copy(ind128[:], iraw[:])
        indT = const.tile([4, 128], F32, tag="indT", name="indT")
        nc.vector.tensor_copy(indT[:], traw[:])
        ones1 = const.tile([128, 128], F32, tag="ones1", name="ones1")
        nc.vector.memset(ones1[:], 1.0)
        onesb = const.tile([128, 32], BF16, tag="onesb", name="onesb")
        nc.vector.memset(onesb[:], 1.0)

        # ---------------- GroupNorm ----------------
        ht = [work.tile([128, N], BF16, tag=f"h{t}", name=f"h{t}") for t in range(CT)]
        st2s, ps_stats = [], []
        for t in range(CT):
            st2 = work.tile([128, 2], F32, tag=f"st2{t}", name=f"st2{t}")
            sc = stage.tile([128, N], F32, tag="gnsc", name="gnsc")
            nc.scalar.activation(sc[:], xt[t][:], AF.Identity, accum_out=st2[:, 0:1])
            nc.scalar.activation(sc[:], xt[t][:], AF.Square, accum_out=st2[:, 1:2])
            ps_stat = ps_av.tile([128, 512], F32, tag=f"av{t}", name=f"gnps{t}")
            nc.tensor.matmul(
                ps_stat[0:4, 0:2], lhsT=ind128[:], rhs=st2[:], start=True, stop=True
            )
            st2s.append(st2)
            ps_stats.append(ps_stat)
        for t in range(CT):
            ps_stat = ps_stats[t]
            sts = work.tile([4, 4], F32, tag=f"gnstat{t}", name=f"gnstat{t}")
            nc.vector.tensor_scalar(
                sts[:, 0:2], ps_stat[0:4, 0:2], 1.0 / (32 * N), None, OP.mult
            )
            nc.vector.tensor_mul(sts[:, 2:3], sts[:, 0:1], sts[:, 0:1])
            nc.vector.tensor_sub(sts[:, 3:4], sts[:, 1:2], sts[:, 2:3])
            nc.vector.tensor_scalar(sts[:, 3:4], sts[:, 3:4], EPS, None, OP.add)
            nc.scalar.activation(sts[:, 2:3], sts[:, 3:4], AF.Sqrt)
            nc.vector.reciprocal(sts[:, 1:2], sts[:, 2:3])
            # one Newton step on rsqrt: r *= 1.5 - 0.5*ve*r^2
            nc.vector.tensor_mul(sts[:, 2:3], sts[:, 1:2], sts[:, 1:2])
            nc.vector.tensor_mul(sts[:, 2:3], sts[:, 2:3], sts[:, 3:4])
            nc.vector.tensor_scalar(sts[:, 2:3], sts[:, 2:3], -0.5, 1.5, OP.mult, OP.add)
            nc.vector.tensor_mul(sts[:, 1:2], sts[:, 1:2], sts[:, 2:3])
            ps_bc = ps_acc.tile([128, 512], F32, tag="den", name=f"gnbc{t}")
            nc.tensor.matmul(
                ps_bc[:, 0:2], lhsT=indT[:], rhs=sts[0:4, 0:2], start=True, stop=True
            )
            chs = work.tile([128, 2], F32, tag=f"chs{t}", name=f"chs{t}")
            nc.vector.tensor_mul(chs[:, 0:1], ps_bc[:, 1:2], gnw[t][:])
            nc.vector.tensor_mul(chs[:, 1:2], ps_bc[:, 0:1], chs[:, 0:1])
            nc.vector.tensor_sub(chs[:, 1:2], gnb[t][:], chs[:, 1:2])
            nc.vector.tensor_scalar(
                ht[t][:], xt[t][:], chs[:, 0:1], chs[:, 1:2], OP.mult, OP.add
            )

        # weight loads + casts (emitted after GN so normalize isn't delayed)
        qkvw = []
        projw = []
        for t in range(CT):
            st = stage.tile([128, 3 * C], F32, tag=f"wstq{t}", name=f"wstq{t}")
            nc.sync.dma_start(out=st[:], in_=ext["qkv_wT"][t])
            w = const.tile([128, 3 * C], BF16, tag=f"qkvw{t}", name=f"qkvw{t}")
            nc.vector.tensor_copy(w[:], st[:])
            qkvw.append(w)
        for t in range(CT):
            st = stage.tile([128, C], F32, tag=f"wstp{t}", name=f"wstp{t}")
            nc.sync.dma_start(out=st[:], in_=ext["proj_wT"][t])
            w = const.tile([128, C], BF16, tag=f"projw{t}", name=f"projw{t}")
            nc.vector.tensor_copy(w[:], st[:])
            projw.append(w)

        # Preload the exp ACT table set so the first real exp does not pay
        # the ~2.7us table switch.
        warm = const.tile([1, 1], F32, tag="warm", name="warm")
        nc.scalar.activation(warm[:], ones1[0:1, 0:1], AF.Exp)

        # ---------------- qkv producers ----------------
        q_sb = [work.tile([128, NQ], BF16, tag=f"q{t}", name=f"q{t}") for t in range(CT)]
        k_sb = [work.tile([128, N], BF16, tag=f"k{t}", name=f"k{t}") for t in range(CT)]
        v_sb = work.tile([128, NMT, NH, HD], BF16, tag="v", name="v")

        # v bias broadcast [128, 256] via ones-matmul
        ps_vb = ps_acc.tile([128, 512], F32, tag="den", name="vbias_ps")
        nc.tensor.matmul(ps_vb[:, 0:C], lhsT=ones1[0:1, :], rhs=vb[:], start=True, stop=True)
        vbias = const.tile([128, C], F32, tag="vbias", name="vbias")
        nc.vector.tensor_copy(vbias[:], ps_vb[:, 0:C])

        def emit_q(qb):
            # q for query block qb (512 cols) into q_sb
            for t in range(CT):
                ps = ps_bank.tile([128, 512], F32, tag="qkv", name="qkv")
                for ct in range(CT):
                    nc.tensor.matmul(
                        ps[:],
                        lhsT=qkvw[ct][:, 128 * t : 128 * (t + 1)],
                        rhs=ht[ct][:, 512 * qb : 512 * (qb + 1)],
                        start=(ct == 0),
                        stop=(ct == 1),
                    )
                nc.vector.tensor_scalar(
                    q_sb[t][:, 512 * qb : 512 * (qb + 1)], ps[:], qb_b[t][:], None, OP.add
                )

        def emit_k(mt):
            # k for key tile mt (128 cols) into k_sb
            ps = ps_bank.tile([128, 512], F32, tag="qkv", name="qkv")
            for t in range(CT):
                for ct in range(CT):
                    nc.tensor.matmul(
                        ps[:, 256 * t : 256 * t + 128],
                        lhsT=qkvw[ct][:, C + 128 * t : C + 128 * (t + 1)],
                        rhs=ht[ct][:, 128 * mt : 128 * (mt + 1)],
                        start=(ct == 0),
                        stop=(ct == 1),
                        skip_group_check=True,
                    )
            for t in range(CT):
                nc.vector.tensor_scalar(
                    k_sb[t][:, 128 * mt : 128 * (mt + 1)],
                    ps[:, 256 * t : 256 * t + 128],
                    kb_b[t][:],
                    None,
                    OP.add,
                )

        def emit_vt(mt):
            ps = ps_bank.tile([128, 512], F32, tag="qkv", name="qkv")
            for ct in range(CT):
                nc.tensor.matmul(
                    ps[:, 0:C],
                    lhsT=ht[ct][:, 128 * mt : 128 * (mt + 1)],
                    rhs=qkvw[ct][:, 2 * C : 3 * C],
                    start=(ct == 0),
                    stop=(ct == 1),
                )
            nc.vector.tensor_add(
                v_sb[:, mt, :, :],
                ps[:, 0:C].rearrange("p (h d) -> p h d", d=HD),
                vbias[:].rearrange("p (h d) -> p h d", d=HD),
            )

        # ---------------- attention ----------------
        # o channel layout (head-interleaved to match av col-tile packing):
        # o_sb[half][row r] = channel 64*(r//32) + 32*half + r%32
        o_sb = [work.tile([128, NQ], BF16, tag=f"o{t}", name=f"o{t}") for t in range(CT)]

        # 8 passes of 256 queries. Score set: bank sA holds heads 0,2 (row
        # tile (0,0)), bank sB heads 1,3 (row tile (64,0)); sets double-
        # buffered so exp(m) overlaps PE work of neighboring iterations.
        # av accumulator bank: [32h:32h+32, 256*dh:...] = head h, dim half dh;
        # den bank columns alternate between passes (manual double buffer).
        ps_den = ps_acc.tile([128, 512], F32, tag="den", name="den")
        ps_avt = [
            ps_av.tile([128, 512], F32, tag=f"av{i}", name=f"av{i}") for i in range(2)
        ]
        for p in range(2 * NQB):
            qcols = slice(256 * p, 256 * (p + 1))
            dcols = slice(256 * (p % 2), 256 * (p % 2) + 256)
            if p % 2 == 0:
                emit_q(p // 2)
            for mt in range(NMT):
                if p == 0:
                    emit_k(mt)
                    emit_vt(mt)
                ps_s = [
                    ps_sc.tile([128, 512], F32, tag=f"s{t}", name=f"s{t}_{p}_{mt}")
                    for t in range(CT)
                ]
                for t in range(CT):
                    for hh in range(2):
                        nc.tensor.matmul(
                            ps_s[hh][:, 256 * t : 256 * (t + 1)],
                            lhsT=k_sb[t][64 * hh : 64 * (hh + 1), 128 * mt : 128 * (mt + 1)],
                            rhs=q_sb[t][64 * hh : 64 * (hh + 1), qcols],
                            start=True,
                            stop=True,
                            skip_group_check=True,
                        )
                # exp: heads 0,2 exact on ScalarE; heads 1,3 single-instruction
                # Schraudolph on DVE (fma -> int16 == bf16 bit pattern)
                pT0 = pp.tile([128, 2, 256], BF16, tag="pT0", name=f"pT0_{p}_{mt}")
                nc.scalar.activation(
                    pT0[:].rearrange("p h q -> p (h q)"), ps_s[0][:], AF.Exp, scale=SCALE
                )
                pT1i = pp.tile([128, 2, 256], BF16, tag="pT1", name=f"pT1_{p}_{mt}")
                if DVE_SCH:
                    nc.vector.tensor_scalar(
                        pT1i[:].bitcast(I16).rearrange("p h q -> p (h q)"),
                        ps_s[1][:], EA, EB, OP.mult, OP.add,
                    )
                else:
                    nc.scalar.activation(
                        pT1i[:].rearrange("p h q -> p (h q)"), ps_s[1][:], AF.Exp,
                        scale=SCALE,
                    )
                pT = [pT0[:], pT1i[:]]
                # pT[0][:, j, :] = head 2j ; pT[1][:, j, :] = head 2j+1
                for dh in range(2):
                    for h in range(NH):
                        nc.tensor.matmul(
                            ps_avt[dh][32 * h : 32 * (h + 1), dcols],
                            lhsT=v_sb[:, mt, h, 32 * dh : 32 * (dh + 1)],
                            rhs=pT[h % 2][:, h // 2, :],
                            start=(mt == 0),
                            stop=(mt == NMT - 1),
                            tile_position=(0, 32 * h),
                            skip_group_check=True,
                        )
                for h in range(NH):
                    nc.tensor.matmul(
                        ps_den[32 * h : 32 * (h + 1), dcols],
                        lhsT=onesb[:],
                        rhs=pT[h % 2][:, h // 2, :],
                        start=(mt == 0),
                        stop=(mt == NMT - 1),
                        tile_position=(0, 32 * h),
                        skip_group_check=True,
                    )
            # normalize: partition i of ps_avb / ps_den belongs to head i//32
            rec = stage.tile([128, 256], F32, tag="rec", name=f"rec_{p}")
            nc.vector.reciprocal(rec[:], ps_den[:, dcols])
            for dh in range(2):
                nc.vector.tensor_mul(
                    o_sb[dh][:, qcols], ps_avt[dh][:, dcols], rec[:]
                )

        if DEBUG_DUMP:
            for t in range(CT):
                nc.sync.dma_start(out=ext["dbg_k"][t], in_=k_sb[t][:])
                nc.sync.dma_start(out=ext["dbg_q"][t], in_=q_sb[t][:])
                nc.sync.dma_start(out=ext["dbg_o"][t], in_=o_sb[t][:])
            nc.sync.dma_start(
                out=ext["dbg_v"][:, :],
                in_=v_sb[:].rearrange("p a b c -> p (a b c)"),
            )

        # ---------------- proj + residual ----------------
        # proj_wT rows are host-permuted to the o_sb head-interleaved order.
        for t in range(CT):
            for nb in range(NQ // 512):
                ps = ps_sc.tile([128, 512], F32, tag=f"s{nb % 2}", name=f"proj{t}_{nb}")
                for ct in range(CT):
                    nc.tensor.matmul(
                        ps[:],
                        lhsT=projw[ct][:, 128 * t : 128 * (t + 1)],
                        rhs=o_sb[ct][:, 512 * nb : 512 * (nb + 1)],
                        start=(ct == 0),
                        stop=(ct == 1),
                    )
                ot = outp.tile([128, 512], F32, tag="out", name="out")
                nc.vector.scalar_tensor_tensor(
                    out=ot[:],
                    in0=ps[:],
                    scalar=projb[t][:],
                    in1=xt[t][:, 512 * nb : 512 * (nb + 1)],
                    op0=OP.add,
                    op1=OP.add,
                )
                nc.sync.dma_start(
                    out=ext["out"][128 * t : 128 * (t + 1), 512 * nb : 512 * (nb + 1)],
                    in_=ot[:],
                )


def _split_multi_waits(nc):
    """Walrus in this container encodes at most ONE semaphore wait per
    engine instruction. Tile emits several. Hoist all-but-one wait of every
    multi-wait instruction into standalone EventSemaphore (wait-only)
    instructions on the same engine stream, which walrus encodes natively.
    Semantically identical (same engine, same program point)."""
    EXEMPT = ("EventSemaphore", "Branch", "Call", "Barrier")
    n_split = 0
    for fn in nc.m.functions:
        for bb in fn.blocks:
            insts = bb.instructions
            out = []
            for inst in insts:
                si = inst.sync_info
                waits = si.on_wait if si is not None and si.on_wait else []
                if len(waits) > 1 and not any(e in type(inst).__name__ for e in EXEMPT):
                    for k, w in enumerate(waits[:-1]):
                        ev = mybir.InstEventSemaphore(
                            name=f"{inst.name}-sw{k}", ins=[], outs=[]
                        )
                        ev.engine = inst.engine
                        ev.sync_info = mybir.SyncInfo(on_wait=[w], on_update=[])
                        out.append(ev)
                    si.on_wait = [waits[-1]]
                    inst.sync_info = si
                    n_split += 1
                out.append(inst)
            if len(out) != len(insts):
                bb.instructions = out
    return n_split


def build_nc(split_waits=True):
    nc = bass.Bass("TRN2", target_bir_lowering=False, debug=False)
    ext = {
        "x": nc.declare_dram_parameter("x", [C, N], F32, isOutput=False),
        "qkv_wT": nc.declare_dram_parameter("qkv_wT", [CT, 128, 3 * C], F32, isOutput=False),
        "qkv_b": nc.declare_dram_parameter("qkv_b", [6, 128, 1], F32, isOutput=False),
        "vb": nc.declare_dram_parameter("vb", [1, C], F32, isOutput=False),
        "proj_wT": nc.declare_dram_parameter("proj_wT", [CT, 128, C], F32, isOutput=False),
        "proj_b": nc.declare_dram_parameter("proj_b", [CT, 128, 1], F32, isOutput=False),
        "gn_w": nc.declare_dram_parameter("gn_w", [CT, 128, 1], F32, isOutput=False),
        "gn_b": nc.declare_dram_parameter("gn_b", [CT, 128, 1], F32, isOutput=False),
        "ind128": nc.declare_dram_parameter("ind128", [128, 4], F32, isOutput=False),
        "indT": nc.declare_dram_parameter("indT", [4, 128], F32, isOutput=False),
        "out": nc.declare_dram_parameter("out", [C, NQ], F32, isOutput=True),
    }
    if DEBUG_DUMP:
        ext["dbg_k"] = nc.declare_dram_parameter("dbg_k", [CT, 128, N], BF16, isOutput=True)
        ext["dbg_q"] = nc.declare_dram_parameter("dbg_q", [CT, 128, NQ], BF16, isOutput=True)
        ext["dbg_o"] = nc.declare_dram_parameter("dbg_o", [CT, 128, NQ], BF16, isOutput=True)
        ext["dbg_v"] = nc.declare_dram_parameter("dbg_v", [128, NMT * NH * HD], BF16, isOutput=True)
    with tile.TileContext(nc) as tc:
        _body(tc, ext)
    if split_waits:
        _split_multi_waits(nc)
    return nc


def make_in_maps(inputs):
    f32 = lambda a: np.ascontiguousarray(np.asarray(a), dtype=np.float32)
    x = f32(inputs["x"]).reshape(B, C, N)
    qkv_wT = f32(np.asarray(inputs["qkv_w"]).T).reshape(CT, 128, 3 * C)
    # proj rows permuted to the o_sb head-interleaved channel order:
    # tile t row r <- channel 64*(r//32) + 32*t + r%32
    proj_w = f32(inputs["proj_w"])
    perm = np.empty((2, 128), np.int64)
    r = np.arange(128)
    for t in range(2):
        perm[t] = 64 * (r // 32) + 32 * t + (r % 32)
    proj_wT_full = np.ascontiguousarray(proj_w.T)  # [in_chan, out_chan]
    proj_wT = np.stack([proj_wT_full[perm[0]], proj_wT_full[perm[1]]], axis=0)
    qkv_b = f32(inputs["qkv_b"]).reshape(6, 128, 1)
    vb = f32(inputs["qkv_b"])[2 * C :].reshape(1, C)
    proj_b = f32(inputs["proj_b"]).reshape(CT, 128, 1)
    gn_w = f32(inputs["gn_w"]).reshape(CT, 128, 1)
    gn_b = f32(inputs["gn_b"]).reshape(CT, 128, 1)
    ind128 = (np.arange(128)[:, None] // 32 == np.arange(4)[None, :]).astype(np.float32)
    indT = np.ascontiguousarray(ind128.T)
    shared = dict(
        qkv_wT=qkv_wT, qkv_b=qkv_b, vb=vb,
        proj_wT=np.ascontiguousarray(proj_wT, dtype=np.float32),
        proj_b=proj_b, gn_w=gn_w, gn_b=gn_b, ind128=ind128, indT=indT,
    )
    in_maps = []
    for c in range(NCORES):
        b, qh = divmod(c, 2)
        xb = x[b]
        if qh:
            xb = np.concatenate([xb[:, NQ:], xb[:, :NQ]], axis=1)
        in_maps.append(dict(x=np.ascontiguousarray(xb), **shared))
    return in_maps


def unshard(results):
    full = np.empty((B, C, N), np.float32)
    for c in range(NCORES):
        b, qh = divmod(c, 2)
        full[b][:, qh * NQ : (qh + 1) * NQ] = results[c]["out"]
    return full.reshape(B, C, 64, 64)


def kernel(**inputs):
    from concourse.bass_utils import run_bass_kernel_spmd

    nc = build_nc()
    res = run_bass_kernel_spmd(nc, make_in_maps(inputs), core_ids=list(range(NCORES)))
    return unshard(res.results)


if __name__ == "__main__":
    nc = build_nc()
    n = sum(len(bb.instructions) for fn in nc.m.functions for bb in fn.blocks)
    print("built ok:", n, "instructions")
